# revision 11
# baseline (speedup 1.0000x reference)
"""Multi-head attention (B=2, N=2048, C=768, H=12) on 8 trn2 cores.

Sharding: core i handles batch b = i//4 and head-group g = i%4 (3 heads each).
All matmul operands are bf16 (host-converted); accumulation is fp32 in PSUM
and the softmax normalization chain is fp32.

Per-core pipeline (q processed in 4 windows of 512 columns):
  1. QKV projection: qT/kT d-major [64, N] per head, v n-major [N, 64] per
     head stored as [1|0*63|v] (softmax denominator ones trick; attn rows
     land at partition base 64, which engine APs require).  PSUM->SBUF
     copies go through ScalarE (idle during the QKV phase) so the DVE
     queue never delays the PSUM ring.
  2. Scores transposed: S^T[k, q] = kT_h chunk.T @ qT_h.  exp via ScalarE
     over kc-PAIRS ([128, 2, 512] PSUM tiles viewed as [128, 1024]).
     Rounds are software-pipelined (scores for pair p+1 emitted before
     attn@V of pair p) so ScalarE never starves; paired heads' score
     matmuls are adjacent with partition bases 0/64 so they run in
     distinct PE row-groups.
  3. attn@V accumulates [denom; 0; attn^T] in one PSUM bank per head.
  4. Normalize after each phase: reciprocal_approx_fast of row 0, gpsimd
     partition broadcast, DVE multiply -> bf16 -> DRAM.
  5. ONE AllGather per window (bf16, 4-core same-batch groups
     [[0,1,2,3],[4,5,6,7]]) triggered at window end; windows run the
     h0/h1 pair phase FIRST and the h2 solo phase second, so the
     previous gather's ~13us gpsimd completion-wait clears before this
     window's first norm needs the queue.  Window 3 splits its gather
     (h0/h1 rows mid-window, h2 rows at the end) so the kernel tail is
     one 64-row gather + a few matmuls.
  6. Output projection column-sharded by w_proj columns; proj(w) runs as
     a compact PE block between window w+1's phases; bias folded in as a
     K=1 matmul.  Output out^T [192, N] fp32; host concat + transpose.

Window 0 interleaves the h2+h1 phases with the QKV chunk groups
(dep-exact) so the PE has no phase boundary and ScalarE gets work early.
"""

import numpy as np

B, N, C, H, HD = 2, 2048, 768, 12, 64
G = 4              # tensor-parallel head groups
HL = H // G        # 3 heads per core
CHL = HL * HD      # 192 local channels
SCALE = HD ** -0.5
NCORES = 8
CT = C // 128      # 6 contraction chunks
FW = 512           # matmul free width == q window width
NWIN = N // FW     # 4 q windows
KT = N // 128      # 16 k chunks (8 pairs)
NP = KT // 2       # 8 kc pairs per (head, window)
KP = G * CHL // 128  # 6 gathered-row chunks (full window gather)
KPA, KPB = 4, 2    # last-window split: heads-0/1 rows, head-2 rows
LW = NWIN - 1      # last window

_CACHE = {}


def _build_nc():
    import concourse.bass as bass  # noqa: F401
    import concourse.bacc as bacc
    import concourse.tile as tile
    import concourse.mybir as mybir

    F32 = mybir.dt.float32
    BF16 = mybir.dt.bfloat16
    AF = mybir.ActivationFunctionType

    nc = bacc.Bacc(num_devices=NCORES)
    xT_d = nc.declare_dram_parameter("xT", [128, CT, N], BF16, isOutput=False)
    wq_d = nc.declare_dram_parameter("wq", [128, CT, CHL], BF16,
                                     isOutput=False)
    wk_d = nc.declare_dram_parameter("wk", [128, CT, CHL], BF16,
                                     isOutput=False)
    wv_d = nc.declare_dram_parameter("wv", [128, CT, CHL], BF16,
                                     isOutput=False)
    wpz_d = nc.declare_dram_parameter("wpz", [128, KP, CHL], BF16,
                                      isOutput=False)
    wpa_d = nc.declare_dram_parameter("wpa", [128, KPA, CHL], BF16,
                                      isOutput=False)
    wpb_d = nc.declare_dram_parameter("wpb", [128, KPB, CHL], BF16,
                                      isOutput=False)
    bp_d = nc.declare_dram_parameter("bp", [1, CHL], BF16, isOutput=False)
    out_d = nc.declare_dram_parameter("out", [CHL, N], F32, isOutput=True)

    RG = [[0, 1, 2, 3], [4, 5, 6, 7]]

    with tile.TileContext(nc) as tc:
        with tc.tile_pool(name="dram", bufs=1, space="DRAM") as dram:
            ag_in = [dram.tile([CHL, FW], BF16, name=f"ag_in{w}")
                     for w in range(LW)]
            ag_out = [dram.tile([G * CHL, FW], BF16, name=f"ag_out{w}")
                      for w in range(LW)]
            aga_in = dram.tile([128, FW], BF16, name="aga_in")
            aga_out = dram.tile([G * 128, FW], BF16, name="aga_out")
            agb_in = dram.tile([64, FW], BF16, name="agb_in")
            agb_out = dram.tile([G * 64, FW], BF16, name="agb_out")

            with tc.tile_pool(name="sb", bufs=1) as P, \
                    tc.tile_pool(name="ps", bufs=1, space="PSUM") as PS, \
                    tc.tile_pool(name="asb", bufs=1) as AS:

                # ---- input DMAs (one per tensor / xT block) ----
                wq_sb = P.tile([128, CT, CHL], BF16)
                wk_sb = P.tile([128, CT, CHL], BF16)
                wv_sb = P.tile([128, CT, CHL], BF16)
                nc.sync.dma_start(out=wk_sb[:], in_=wk_d[:, :, :])
                nc.sync.dma_start(out=wq_sb[:], in_=wq_d[:, :, :])
                xT_sb = P.tile([128, CT, N], BF16)
                for blk in range(NWIN):
                    nc.sync.dma_start(
                        out=xT_sb[:, :, blk * FW:(blk + 1) * FW],
                        in_=xT_d[:, :, blk * FW:(blk + 1) * FW],
                    )
                nc.sync.dma_start(out=wv_sb[:], in_=wv_d[:, :, :])
                wpz_sb = P.tile([128, KP, CHL], BF16)
                wpa_sb = P.tile([128, KPA, CHL], BF16)
                wpb_sb = P.tile([128, KPB, CHL], BF16)
                nc.sync.dma_start(out=wpz_sb[:], in_=wpz_d[:, :, :])
                nc.sync.dma_start(out=wpa_sb[:], in_=wpa_d[:, :, :])
                nc.sync.dma_start(out=wpb_sb[:], in_=wpb_d[:, :, :])
                bp_sb = P.tile([1, CHL], BF16)
                nc.sync.dma_start(out=bp_sb[:], in_=bp_d[:, :])
                ones_q = P.tile([1, FW], BF16)
                nc.vector.memset(ones_q[:], 1.0)

                # ---- persistent QKV results (bf16) ----
                q01 = P.tile([128, N], BF16)   # qT heads 0,1
                q2 = P.tile([64, N], BF16)     # qT head 2
                k01 = P.tile([128, N], BF16)
                k2 = P.tile([64, N], BF16)
                # [kpos, kc, h, 128]: col 0 = ones, 1:64 zeros, 64:128 = v
                v_sb = P.tile([128, KT, HL, 2 * HD], BF16)
                nc.vector.memset(v_sb[:, :, :, 0:1], 1.0)
                nc.vector.memset(v_sb[:, :, :, 1:HD], 0.0)

                QH = (q01[0:64], q01[64:128], q2[0:64])
                KH = (k01[0:64], k01[64:128], k2[0:64])

                def emit_qk(f):
                    for dst, wsb, mlo, mhi in (
                        (k2, wk_sb, 128, CHL),
                        (q2, wq_sb, 128, CHL),
                        (k01, wk_sb, 0, 128),
                        (q01, wq_sb, 0, 128),
                    ):
                        m = mhi - mlo
                        ps_t = PS.tile([m, FW], F32, tag="mm", bufs=2,
                                       padded_shape=[128, FW], name="qk_ps")
                        for ct in range(CT):
                            nc.tensor.matmul(
                                ps_t[:],
                                lhsT=wsb[:, ct, mlo:mhi],
                                rhs=xT_sb[:, ct, f * FW:(f + 1) * FW],
                                start=(ct == 0), stop=(ct == CT - 1),
                            )
                        nc.scalar.copy(dst[:, f * FW:(f + 1) * FW], ps_t[:])

                def emit_v(nt):
                    ps_t = PS.tile([128, CHL], F32, tag="mm", bufs=2,
                                   padded_shape=[128, FW], name="v_ps")
                    for ct in range(CT):
                        nc.tensor.matmul(
                            ps_t[:],
                            lhsT=xT_sb[:, ct, nt * 128:(nt + 1) * 128],
                            rhs=wv_sb[:, ct, :],
                            start=(ct == 0), stop=(ct == CT - 1),
                        )
                    nc.scalar.copy(
                        v_sb[:, nt, :, HD:2 * HD],
                        ps_t[:].rearrange("p (h d) -> p h d", h=HL))

                def new_A():
                    return PS.tile([128, FW], F32, tag="A", bufs=2, name="A")

                def score_mms(w, hs, p):
                    Ss = {h: PS.tile([128, 2, FW], F32, tag="S", bufs=2,
                                     name="S") for h in hs}
                    for j in range(2):
                        kc = 2 * p + j
                        for h in hs:
                            nc.tensor.matmul(
                                Ss[h][:, j, :],
                                lhsT=KH[h][:, kc * 128:(kc + 1) * 128],
                                rhs=QH[h][:, w * FW:(w + 1) * FW],
                            )
                    return Ss

                def exp_mms(Ss, hs):
                    Es = {}
                    for h in hs:
                        E = AS.tile([128, 2, FW], BF16, tag="E", bufs=4,
                                    name="E")
                        nc.scalar.activation(E[:, :, :], Ss[h][:, :, :],
                                             AF.Exp, scale=SCALE)
                        Es[h] = E
                    return Es

                def av_mms(hs, p, Es, As):
                    for h in hs:
                        for j in range(2):
                            kc = 2 * p + j
                            nc.tensor.matmul(
                                As[h][:],
                                lhsT=v_sb[:, kc, h, :],
                                rhs=Es[h][:, j, :],
                                start=(p == 0 and j == 0),
                                stop=(p == NP - 1 and j == 1),
                            )

                def norm_store(w, h, A):
                    R = AS.tile([1, FW], F32, tag="R", bufs=2, name="R")
                    bcs = AS.tile([128, FW], F32, tag="bcs", bufs=2,
                                  name="bcs")
                    attn_t = AS.tile([128, FW], BF16, tag="attn", bufs=3,
                                     name="attn_t")
                    nc.vector.reciprocal_approx_fast(out=R[:], in_=A[0:1, :])
                    nc.gpsimd.partition_broadcast(bcs[:], R[0:1, :])
                    nc.vector.tensor_mul(attn_t[64:128, :], A[64:128, :],
                                         bcs[64:128, :])
                    if w == LW:
                        if h == 2:
                            nc.sync.dma_start(out=agb_in[:, :],
                                              in_=attn_t[64:128, :])
                        else:
                            nc.sync.dma_start(
                                out=aga_in[h * HD:(h + 1) * HD, :],
                                in_=attn_t[64:128, :])
                    else:
                        nc.sync.dma_start(
                            out=ag_in[w][h * HD:(h + 1) * HD, :],
                            in_=attn_t[64:128, :])

                def gather(w):
                    nc.gpsimd.collective_compute(
                        "AllGather", mybir.AluOpType.bypass,
                        replica_groups=RG,
                        ins=[ag_in[w].opt()], outs=[ag_out[w].opt()])

                def gather_ab(which):
                    i, o = ((aga_in, aga_out) if which == "a"
                            else (agb_in, agb_out))
                    nc.gpsimd.collective_compute(
                        "AllGather", mybir.AluOpType.bypass,
                        replica_groups=RG, ins=[i.opt()], outs=[o.opt()])

                def att_phase(w, hs, As, fillers=None):
                    """Software-pipelined rounds; norms at the end."""
                    Ss = score_mms(w, hs, 0)
                    Es = exp_mms(Ss, hs)
                    for p in range(NP):
                        if p + 1 < NP:
                            Sn = score_mms(w, hs, p + 1)
                            En = exp_mms(Sn, hs)
                        av_mms(hs, p, Es, As)
                        if fillers is not None and p in fillers:
                            fillers[p]()
                        if p + 1 < NP:
                            Es = En
                    for h in hs:
                        norm_store(w, h, As[h])

                # -- proj(w) for full-window gathers (w = 0..2) --
                proj_st = {}

                def proj_dma(w):
                    ao = AS.tile([128, KP, FW], BF16, tag="ao", bufs=2,
                                 name="ao")
                    for kp in range(KP):
                        nc.sync.dma_start(
                            out=ao[:, kp, :],
                            in_=ag_out[w][kp * 128:(kp + 1) * 128, :])
                    proj_st[w] = ao

                def proj_mms(w):
                    ao = proj_st[w]
                    for mlo, mhi in ((0, 128), (128, CHL)):
                        pr = PS.tile([mhi - mlo, FW], F32, tag="mm", bufs=2,
                                     padded_shape=[128, FW], name="pr")
                        for kp in range(KP):
                            nc.tensor.matmul(
                                pr[:], lhsT=wpz_sb[:, kp, mlo:mhi],
                                rhs=ao[:, kp, :], start=(kp == 0), stop=False)
                        nc.tensor.matmul(
                            pr[:], lhsT=bp_sb[:, mlo:mhi], rhs=ones_q[:],
                            start=False, stop=True)
                        o_t = AS.tile([mhi - mlo, FW], F32, tag="o", bufs=2,
                                      padded_shape=[128, FW], name="o_t")
                        nc.vector.tensor_copy(o_t[:], pr[:])
                        nc.sync.dma_start(
                            out=out_d[mlo:mhi, w * FW:(w + 1) * FW],
                            in_=o_t[:])

                def proj_last():
                    """proj(LW): heads-0/1 part (gather a), then head-2
                    part (gather b) + bias; only part b is in the tail."""
                    aoa = AS.tile([128, KPA, FW], BF16, tag="ao", bufs=2,
                                  name="aoa")
                    for kp in range(KPA):
                        nc.sync.dma_start(
                            out=aoa[:, kp, :],
                            in_=aga_out[kp * 128:(kp + 1) * 128, :])
                    aob = AS.tile([128, KPB, FW], BF16, tag="ao3", bufs=1,
                                  name="aob")
                    for kp in range(KPB):
                        nc.sync.dma_start(
                            out=aob[:, kp, :],
                            in_=agb_out[kp * 128:(kp + 1) * 128, :])
                    for mlo, mhi in ((0, 128), (128, CHL)):
                        pr = PS.tile([mhi - mlo, FW], F32, tag="mm", bufs=2,
                                     padded_shape=[128, FW], name="pr3")
                        for kp in range(KPA):
                            nc.tensor.matmul(
                                pr[:], lhsT=wpa_sb[:, kp, mlo:mhi],
                                rhs=aoa[:, kp, :], start=(kp == 0),
                                stop=False)
                        for kp in range(KPB):
                            nc.tensor.matmul(
                                pr[:], lhsT=wpb_sb[:, kp, mlo:mhi],
                                rhs=aob[:, kp, :], start=False, stop=False)
                        nc.tensor.matmul(
                            pr[:], lhsT=bp_sb[:, mlo:mhi], rhs=ones_q[:],
                            start=False, stop=True)
                        o_t = AS.tile([mhi - mlo, FW], F32, tag="o", bufs=2,
                                      padded_shape=[128, FW], name="o_t3")
                        nc.vector.tensor_copy(o_t[:], pr[:])
                        nc.sync.dma_start(
                            out=out_d[mlo:mhi, LW * FW:(LW + 1) * FW],
                            in_=o_t[:])

                # ----------------- emission schedule -----------------
                # Window 0: h2+h1 rounds software-pipelined and interleaved
                # with QKV chunk groups (pair p needs k block p//2 and v
                # chunks 2p, 2p+1); then h0 solo; one gather at window end.
                A2, A1 = new_A(), new_A()
                emit_qk(0)
                emit_v(0); emit_v(1)
                w0_fill = {
                    0: lambda: (emit_qk(1), emit_v(2), emit_v(3)),
                    1: lambda: (emit_v(4), emit_v(5)),
                    2: lambda: (emit_qk(2), emit_v(6), emit_v(7)),
                    3: lambda: (emit_v(8), emit_v(9)),
                    4: lambda: (emit_qk(3), emit_v(10), emit_v(11)),
                    5: lambda: (emit_v(12), emit_v(13)),
                    6: lambda: (emit_v(14), emit_v(15)),
                }
                Ss = score_mms(0, [2, 1], 0)
                Es = exp_mms(Ss, [2, 1])
                for p in range(NP):
                    if p in w0_fill:
                        w0_fill[p]()
                    if p + 1 < NP:
                        Sn = score_mms(0, [2, 1], p + 1)
                        En = exp_mms(Sn, [2, 1])
                    av_mms([2, 1], p, Es, {2: A2, 1: A1})
                    if p + 1 < NP:
                        Es = En
                norm_store(0, 2, A2)
                norm_store(0, 1, A1)
                A0 = new_A()
                att_phase(0, [0], {0: A0})
                gather(0)

                # Windows 1, 2: h0/h1 pair phase first (issues proj DMAs
                # early), then proj(w-1) matmul block, then h2 solo phase;
                # single gather at window end.
                for w in (1, 2):
                    A0, A1 = new_A(), new_A()
                    att_phase(w, [0, 1], {0: A0, 1: A1},
                              fillers={1: (lambda wp=w - 1: proj_dma(wp))})
                    proj_mms(w - 1)
                    A2 = new_A()
                    att_phase(w, [2], {2: A2})
                    gather(w)

                # Window 3: pair phase -> gather a (h0/h1 rows); proj(2)
                # block; h2 phase -> gather b; proj(3) tail.
                A0, A1 = new_A(), new_A()
                att_phase(LW, [0, 1], {0: A0, 1: A1},
                          fillers={1: lambda: proj_dma(2)})
                gather_ab("a")
                proj_mms(2)
                A2 = new_A()
                att_phase(LW, [2], {2: A2})
                gather_ab("b")
                proj_last()
    nc.finalize()
    return nc


def get_nc():
    if "nc" not in _CACHE:
        _CACHE["nc"] = _build_nc()
    return _CACHE["nc"]


def _pack128(a):
    """[K, M] -> [128, K//128, M] partition-major packing."""
    from ml_dtypes import bfloat16

    k, m = a.shape
    return np.ascontiguousarray(
        a.reshape(k // 128, 128, m).transpose(1, 0, 2)).astype(bfloat16)


def make_in_maps(x, w_qkv, w_proj, b_proj):
    from ml_dtypes import bfloat16

    x = np.asarray(x, dtype=np.float32)
    w_qkv = np.asarray(w_qkv, dtype=np.float32)
    w_proj = np.asarray(w_proj, dtype=np.float32)
    b_proj = np.asarray(b_proj, dtype=np.float32)
    # last-window split row orders (rank-major):
    idx_a = np.concatenate(
        [np.arange(192 * r, 192 * r + 128) for r in range(G)])
    idx_b = np.concatenate(
        [np.arange(192 * r + 128, 192 * (r + 1)) for r in range(G)])
    in_maps = []
    for core in range(NCORES):
        b, g = divmod(core, G)
        cs = slice(g * CHL, (g + 1) * CHL)
        wp = w_proj[:, cs]
        im = {
            "xT": _pack128(np.ascontiguousarray(x[b].T)),
            "wq": _pack128(np.ascontiguousarray(w_qkv[:, 0 * C:1 * C][:, cs])),
            "wk": _pack128(np.ascontiguousarray(w_qkv[:, 1 * C:2 * C][:, cs])),
            "wv": _pack128(np.ascontiguousarray(w_qkv[:, 2 * C:3 * C][:, cs])),
            "wpz": _pack128(np.ascontiguousarray(wp)),
            "wpa": _pack128(np.ascontiguousarray(wp[idx_a])),
            "wpb": _pack128(np.ascontiguousarray(wp[idx_b])),
            "bp": np.ascontiguousarray(
                b_proj[cs].reshape(1, CHL)).astype(bfloat16),
        }
        in_maps.append(im)
    return in_maps


def unshard(results):
    out = np.empty((B, N, C), dtype=np.float32)
    for b in range(B):
        outT = np.concatenate(
            [results[b * G + g]["out"] for g in range(G)], axis=0)
        out[b] = outT.T
    return out


def kernel(x, w_qkv, w_proj, b_proj):
    from concourse.bass_utils import run_bass_kernel_spmd

    nc = get_nc()
    in_maps = make_in_maps(x, w_qkv, w_proj, b_proj)
    res = run_bass_kernel_spmd(nc, in_maps, list(range(NCORES)))
    return unshard(res.results)
